# revision 1
# baseline (speedup 1.0000x reference)
"""Trainium2 Bass kernel for the CPC/moe_routing problem.

Strategy: the problem fully decomposes by category (the [N,N] negative-term
matrix is only needed where c_i == c_j).  We shard BY CATEGORY: 16 categories
across 8 cores = 2 categories/core.  Each core computes, for its rows only:
  f_x = relu(x@W1+b1)@W2+b2, f_z = z@Wz+bz, u = f_x @ w_s[cat]
  S = softplus(u @ f_z^T) per category block, neg_T = row-mean over the
  category, T = softplus(diag) via elementwise u*f_z,
  out = log(T+eps) - log(neg_T+eps)
On-chip layouts are transposed ([feature, row]) so matmuls contract along
partitions and biases are per-partition.  Matmul operands are fp16 (weights
host-rounded; activations device-rounded) with fp32 PSUM accumulation; the
second MLP layer is host-fused with the routing weights (W2c = W2 @ w_s[g]).

Numerical notes:
- negative-term sum uses softplus(v) ~= relu(v): with per-row |v| std >= 10
  on these inputs the dropped log1p(exp(-|v|)) term biases neg_T by <= 6e-3
  (~1e-4 relative), i.e. <~1e-3 absolute on the final log output.
- rows padded up to the per-category capacity P get z := z0 with
  z0 = -Wz^-T bz (host-solved), so their f_z is ~0 on device and they
  contribute ~nothing to the relu-sum; counts use the true 1/cnt from host.
- the positive term log(softplus(pos)+eps) is computed with an exact
  piecewise form (it is sensitive when pos is very negative).
"""

import math
from contextlib import ExitStack

import numpy as np

import concourse.bass as bass
import concourse.mybir as mybir
import concourse.tile as tile
from concourse import bacc
from concourse import bass_utils

F32 = mybir.dt.float32
F32R = mybir.dt.float32r
BF16 = mybir.dt.bfloat16
FP16 = mybir.dt.float16
AF = mybir.ActivationFunctionType
ALU = mybir.AluOpType

N, D_IN, HID, Z, C = 8192, 256, 512, 128, 16
N_CORES = 8
CATS_PER_CORE = C // N_CORES
EPS32 = float(np.float32(1e-16))
LNEPS = float(np.log(np.float64(np.float32(1e-16))))  # -36.8413614...
POS_THRESH = -9.0
N_WARMUP_MM = 28


def _col_tiles(total, step=512):
    tiles = []
    s = 0
    while s < total:
        nt = min(step, total - s)
        tiles.append((s, nt))
        s += nt
    return tiles


def build_program(P):
    """Build the single-core Bass/Tile program (SPMD: same NEFF on all cores)."""
    NCH = P // 128
    R = CATS_PER_CORE * P
    F = R // 128  # chunk-major columns of per-row [128, F] vectors
    TIL = _col_tiles(P)
    RTIL = _col_tiles(R)

    nc = bacc.Bacc(
        "TRN2",
        target_bir_lowering=False,
        debug=False,
        enable_asserts=False,
        num_devices=N_CORES,
    )

    xT = nc.dram_tensor("xT", [2, 128, R], FP16, kind="ExternalInput")
    zT = nc.dram_tensor("zT", [128, R], FP16, kind="ExternalInput")
    W1 = nc.dram_tensor("W1", [2, 128, HID], FP16, kind="ExternalInput")
    W2c = nc.dram_tensor("W2c", [CATS_PER_CORE, 4, 128, Z], FP16, kind="ExternalInput")
    Wz = nc.dram_tensor("Wz", [Z, Z], FP16, kind="ExternalInput")
    b1 = nc.dram_tensor("b1", [128, 4], F32, kind="ExternalInput")
    b2c = nc.dram_tensor("b2c", [128, CATS_PER_CORE], F32, kind="ExternalInput")
    bz = nc.dram_tensor("bz", [128, 1], F32, kind="ExternalInput")
    cstd = nc.dram_tensor("cst", [128, 1], F32R, kind="ExternalInput")
    invd = nc.dram_tensor("invd", [128, F], F32, kind="ExternalInput")
    outd = nc.dram_tensor("out", [128, F], F32, kind="ExternalOutput")

    with tile.TileContext(nc) as tc, ExitStack() as ctx:
        perm = ctx.enter_context(tc.tile_pool(name="perm", bufs=1))
        vec = ctx.enter_context(tc.tile_pool(name="vec", bufs=1))

        # ---- PE warm-up: keep the HAM activity monitor busy while DMAs run,
        # so real matmuls start (and stay) at 2.4 GHz instead of 1.2 GHz.
        with (
            tc.tile_pool(name="warm", bufs=1) as warm,
            tc.tile_pool(name="pswarm", bufs=1, space="PSUM") as pswarm,
        ):
            wdum = warm.tile([128, 256], BF16)
            nc.gpsimd.memset(wdum[:], 0.5)
            pdum = pswarm.tile([16, 256], F32)
            for _ in range(N_WARMUP_MM):
                nc.tensor.matmul(
                    pdum[:], wdum[:, 0:16], wdum[:], start=True, stop=True
                )

        # ---- persistent weights / constants ----
        # W1/b1 first: the first row-tile's matmuls only need these, so the
        # PE can start while the rest of the weights stream in.
        sbW1 = perm.tile([128, 2, HID], FP16)
        for f in range(2):
            nc.scalar.dma_start(sbW1[:, f, :], W1[f])
        sbb1 = perm.tile([128, 4], F32)
        nc.scalar.dma_start(sbb1[:], b1[:])
        sbW2c = perm.tile([128, CATS_PER_CORE, 4, Z], FP16)
        for g in range(CATS_PER_CORE):
            for q in range(4):
                nc.scalar.dma_start(sbW2c[:, g, q, :], W2c[g, q])
        sbb2c = perm.tile([128, CATS_PER_CORE], F32)
        nc.scalar.dma_start(sbb2c[:], b2c[:])
        sbWz = perm.tile([128, Z], FP16)
        sbbz = perm.tile([128, 1], F32)
        sbones = perm.tile([128, 1], F32R)
        sbinv = perm.tile([128, F], F32)
        sbeps = perm.tile([128, 1], F32)
        nc.gpsimd.memset(sbeps[:], EPS32)

        def load_rest_of_weights():
            nc.scalar.dma_start(sbWz[:], Wz[:])
            nc.scalar.dma_start(sbbz[:], bz[:])
            nc.scalar.dma_start(sbones[:], cstd[:])
            nc.scalar.dma_start(sbinv[:], invd[:])

        # ---- persistent activations ----
        sbfz = perm.tile([128, R], F32R)
        sbfzh = perm.tile([128, R], FP16)
        sbu = perm.tile([128, R], FP16)
        sbprod = perm.tile([128, R], F32R)
        nacc = perm.tile([128, F], F32)  # per-row relu-sum accumulators

        load_rest_of_weights()

        # ======== Stage B: MLP + f_z over row tiles; u per category ========
        with (
            tc.tile_pool(name="xin", bufs=4) as xin,
            tc.tile_pool(name="hrelu", bufs=2) as hpool,
            tc.tile_pool(name="psB", bufs=1, space="PSUM") as psB,
            tc.tile_pool(name="psB1", bufs=1, space="PSUM") as psB1,
            tc.tile_pool(name="psp", bufs=1, space="PSUM") as psp,
        ):
            pspos = psp.tile([128, F], F32)
            for (ts, nt) in RTIL:
                sl = slice(ts, ts + nt)
                xt = xin.tile([128, 2, nt], FP16, tag="xt")
                for f in range(2):
                    nc.sync.dma_start(xt[:, f, :], xT[f, :, sl])
                zt = xin.tile([128, nt], FP16, tag="zt")
                nc.sync.dma_start(zt[:], zT[:, sl])

                ph = psB.tile([128, 4, nt], F32, tag="ph")
                for h in range(4):
                    hs = slice(h * 128, (h + 1) * 128)
                    for f in range(2):
                        nc.tensor.matmul(
                            ph[:, h, :],
                            sbW1[:, f, hs],
                            xt[:, f, :],
                            start=(f == 0),
                            stop=(f == 1),
                        )
                ht = hpool.tile([128, 4, nt], FP16, tag="ht")
                for h in range(4):
                    # ht = relu(ph + b1)  (ACT: per-partition bias is free)
                    nc.scalar.activation(
                        ht[:, h, :], ph[:, h, :], AF.Relu, bias=sbb1[:, h : h + 1]
                    )

                pfz = psB1.tile([128, nt], F32, tag="pfz", bufs=2)
                nc.tensor.matmul(pfz[:], sbWz[:], zt[:], start=True, stop=True)
                nc.vector.tensor_scalar_add(sbfz[:, sl], pfz[:], sbbz[:, 0:1])
                nc.vector.tensor_scalar_add(sbfzh[:, sl], pfz[:], sbbz[:, 0:1])

                # u directly from h via the host-fused W2c = W2 @ w_s[cat]
                # (split the row range at category boundaries).  The
                # positive-term pos[p, c] = prod[:, c*128+p] . ones lands
                # directly in chunk-major [128, F] layout by using the prod
                # block as the STATIONARY operand.
                s0 = ts
                while s0 < ts + nt:
                    g = s0 // P
                    e0 = min(ts + nt, (g + 1) * P)
                    cn = e0 - s0
                    slc = slice(s0, e0)
                    pu = psB1.tile([128, cn], F32, tag="pu", name=f"pu_{s0}")
                    for q in range(4):
                        nc.tensor.matmul(
                            pu[:],
                            sbW2c[:, g, q, :],
                            ht[:, q, s0 - ts : e0 - ts],
                            start=(q == 0),
                            stop=(q == 3),
                        )
                    b2g = sbb2c[:, g : g + 1]
                    nc.vector.tensor_scalar_add(sbu[:, slc], pu[:], b2g)
                    nc.vector.scalar_tensor_tensor(
                        sbprod[:, slc], pu[:], b2g, sbfz[:, slc],
                        op0=ALU.add, op1=ALU.mult,
                    )
                    for cc in range(cn // 128):
                        col = s0 // 128 + cc
                        c0 = s0 + cc * 128
                        # N=1 violates fp32r ISA rules; plain fp32 is fine
                        # here (cost is the ~60-cycle floor anyway)
                        nc.tensor.matmul(
                            pspos[:, col : col + 1],
                            sbprod[:, c0 : c0 + 128].bitcast(F32),
                            sbones[:].bitcast(F32),
                            start=True, stop=True,
                        )
                    s0 = e0

            tpos = vec.tile([128, F], F32)
            nc.vector.tensor_copy(tpos[:], pspos[:])

        # ======== positive-term log-space chain (overlaps the neg loop) =====

        # ACT set 1 (exp_and_others: Abs/Exp), then set 2 (natural_log: Ln)
        t_ax = vec.tile([128, F], F32)
        i_ax = nc.scalar.activation(t_ax[:], tpos[:], AF.Abs)
        t_y = vec.tile([128, F], F32)
        nc.vector.tensor_scalar_add(t_y[:], tpos[:], -LNEPS)
        t_ay = vec.tile([128, F], F32)
        i_ay = nc.scalar.activation(t_ay[:], t_y[:], AF.Abs)
        t_e2 = vec.tile([128, F], F32)
        i_e2 = nc.scalar.activation(t_e2[:], t_ax[:], AF.Exp, scale=-1.0)
        t_e1 = vec.tile([128, F], F32)
        i_e1 = nc.scalar.activation(t_e1[:], t_ay[:], AF.Exp, scale=-1.0)
        t_r2 = vec.tile([128, F], F32)
        nc.vector.tensor_scalar_max(t_r2[:], tpos[:], 0.0)
        t_r1 = vec.tile([128, F], F32)
        nc.vector.tensor_scalar_max(t_r1[:], t_y[:], 0.0)
        t_l2 = vec.tile([128, F], F32)
        i_l2 = nc.scalar.activation(t_l2[:], t_e2[:], AF.Ln, bias=1.0)
        t_l1 = vec.tile([128, F], F32)
        i_l1 = nc.scalar.activation(t_l1[:], t_e1[:], AF.Ln, bias=1.0)
        # batch ACT ops by table set: Abs/Exp (resident set), then the Lns
        tile.add_dep_helper(i_e2.ins, i_ay.ins, sync=False, reason="act batch")
        tile.add_dep_helper(i_l2.ins, i_e1.ins, sync=False, reason="act batch")
        t_sp = vec.tile([128, F], F32)
        nc.vector.tensor_add(t_sp[:], t_r2[:], t_l2[:])
        t_p2 = vec.tile([128, F], F32)
        i_p2 = nc.scalar.activation(t_p2[:], t_sp[:], AF.Ln, bias=sbeps[:])
        tile.add_dep_helper(i_p2.ins, i_l1.ins, sync=False, reason="act batch")
        t_p1 = vec.tile([128, F], F32)
        nc.vector.scalar_tensor_tensor(
            t_p1[:], t_r1[:], LNEPS, t_l1[:], op0=ALU.add, op1=ALU.add
        )
        t_m = vec.tile([128, F], mybir.dt.int32)
        nc.vector.tensor_scalar(t_m[:], tpos[:], POS_THRESH, None, op0=ALU.is_lt)
        t_posln = vec.tile([128, F], F32)
        nc.vector.select(t_posln[:], t_m[:], t_p1[:], t_p2[:])

        # ======== Stage C: negative sums ========
        with (
            tc.tile_pool(name="junkp", bufs=2) as jpool,
            tc.tile_pool(name="psm", bufs=2, space="PSUM") as psm,
        ):
            # per category, per 128-row i-chunk:
            #   M'[i, j] = u_i . f_z_j for all j; nacc[:, chunk] = sum_j relu
            for g in range(CATS_PER_CORE):
                for ic in range(NCH):
                    ucol = g * P + ic * 128
                    pm = psm.tile([128, P], F32, tag="pm")
                    for (ts, nt) in TIL:
                        nc.tensor.matmul(
                            pm[:, ts : ts + nt],
                            sbu[:, ucol : ucol + 128],
                            sbfzh[:, g * P + ts : g * P + ts + nt],
                            start=True, stop=True,
                        )
                    junk = jpool.tile([128, P], F32, tag="junk")
                    col = g * NCH + ic
                    nc.vector.tensor_scalar(
                        junk[:], pm[:], 0.0, 0.0, op0=ALU.max, op1=ALU.add,
                        accum_out=nacc[:, col : col + 1],
                    )


        # ======== final combination ========
        t_negT = vec.tile([128, F], F32)
        nc.vector.tensor_mul(t_negT[:], nacc[:], sbinv[:])
        t_lnneg = vec.tile([128, F], F32)
        i_lnneg = nc.scalar.activation(t_lnneg[:], t_negT[:], AF.Ln, bias=sbeps[:])
        # keep the Ln-set ops together: lnneg must not jump ahead of the
        # pos-chain Lns or the ACT table set gets reloaded twice
        tile.add_dep_helper(
            i_lnneg.ins, i_p2.ins, sync=False, reason="act table order"
        )

        t_out = vec.tile([128, F], F32)
        nc.vector.tensor_sub(t_out[:], t_posln[:], t_lnneg[:])
        nc.sync.dma_start(outd[:], t_out[:])

    nc.compile()
    return nc


def prepare(x, c, z, W1, b1, W2, b2, Wz, bz, w_s):
    """Host-side sharding: returns (P, in_maps, slots, idx)."""
    x = np.ascontiguousarray(np.asarray(x, dtype=np.float32))
    z = np.ascontiguousarray(np.asarray(z, dtype=np.float32))
    W1 = np.asarray(W1, dtype=np.float32)
    b1 = np.asarray(b1, dtype=np.float32)
    W2 = np.asarray(W2, dtype=np.float32)
    b2 = np.asarray(b2, dtype=np.float32)
    Wz = np.asarray(Wz, dtype=np.float32)
    bz = np.asarray(bz, dtype=np.float32)
    w_s = np.asarray(w_s, dtype=np.float32)
    ci = np.asarray(c).astype(np.int64)

    idx = [np.nonzero(ci == g)[0] for g in range(C)]
    cnt = np.array([len(i) for i in idx])
    P = 128 * max(1, math.ceil(cnt.max() / 128))
    NCH = P // 128
    R = CATS_PER_CORE * P
    F = R // 128

    # padded rows get z0 with Wz^T z0 + bz = 0, so their f_z vanishes on
    # device (solve against the fp16-rounded Wz the device actually uses)
    z0 = -np.linalg.solve(
        Wz.astype(np.float16).astype(np.float64).T, bz.astype(np.float64)
    )
    z0 = z0.astype(np.float32)

    W1h = np.ascontiguousarray(W1.reshape(2, 128, HID).astype(np.float16))
    b1h = np.ascontiguousarray(b1.reshape(4, 128).T)  # [128, 4]
    bzh = np.ascontiguousarray(bz.reshape(128, 1))
    cst_arr = np.ones((128, 1), dtype=np.float32)
    Wzh = np.ascontiguousarray(Wz.astype(np.float16))
    # host-fused second layer: W2c[g] = W2 @ w_s[g], b2c[g] = b2 @ w_s[g]
    W2c_all = np.einsum(
        "hd,cde->che", W2.astype(np.float64), w_s.astype(np.float64)
    )  # [C, HID, Z]
    b2c_all = np.einsum(
        "d,cde->ce", b2.astype(np.float64), w_s.astype(np.float64)
    )  # [C, Z]

    in_maps = []
    slots = []
    for k in range(N_CORES):
        cats = [CATS_PER_CORE * k + j for j in range(CATS_PER_CORE)]
        padded = []
        inv_chunk = np.zeros((128, F), dtype=np.float32)
        pad_flags = np.zeros(R, dtype=bool)
        for j, g in enumerate(cats):
            n_real = cnt[g]
            pad_to = P - n_real
            fill = idx[g][0] if n_real > 0 else 0
            padded.append(
                np.concatenate([idx[g], np.full(pad_to, fill, dtype=idx[g].dtype)])
            )
            pad_flags[j * P + n_real : (j + 1) * P] = True
            inv_chunk[:, j * NCH : (j + 1) * NCH] = 1.0 / max(n_real, 1)
        rows = np.concatenate(padded)  # [R] global row indices
        xTk = np.ascontiguousarray(x[rows].T.reshape(2, 128, R).astype(np.float16))
        zk = z[rows].copy()
        zk[pad_flags] = z0[None, :, 0] if z0.ndim == 2 else z0
        zTk = np.ascontiguousarray(zk.T.astype(np.float16))
        W2ck = np.ascontiguousarray(
            W2c_all[cats].reshape(CATS_PER_CORE, 4, 128, Z).astype(np.float16)
        )
        b2ck = np.ascontiguousarray(
            b2c_all[cats].T.astype(np.float32)
        )  # [128, CATS_PER_CORE]
        in_maps.append(
            {
                "xT": xTk,
                "zT": zTk,
                "W1": W1h,
                "W2c": W2ck,
                "Wz": Wzh,
                "b1": b1h,
                "b2c": b2ck,
                "bz": bzh,
                "cst": cst_arr,
                "invd": inv_chunk,
            }
        )
        slots.append((cats, [cnt[g] for g in cats]))
    return P, in_maps, slots, idx


def gather_output(P, slots, idx, core_outs):
    NCH = P // 128
    out_full = np.zeros(N, dtype=np.float32)
    for k in range(N_CORES):
        om = core_outs[k]  # [128, F], out[p, g*NCH+r] = row g*P + r*128 + p
        cats, counts = slots[k]
        for j, g in enumerate(cats):
            rows_cat = om[:, j * NCH : (j + 1) * NCH].T.reshape(P)
            n_real = counts[j]
            if n_real:
                out_full[idx[g]] = rows_cat[:n_real]
    return out_full


def kernel(x, c, z, W1, b1, W2, b2, Wz, bz, w_s):
    P, in_maps, slots, idx = prepare(x, c, z, W1, b1, W2, b2, Wz, bz, w_s)
    nc = build_program(P)
    res = bass_utils.run_bass_kernel_spmd(nc, in_maps, core_ids=list(range(N_CORES)))
    return gather_output(P, slots, idx, [r["out"] for r in res.results])



# revision 8
# speedup vs baseline: 1.2355x; 1.2355x over previous
"""Trainium2 Bass kernel for the CPC/moe_routing problem (v2).

Strategy (category sharding, no collectives): the [N,N] negative-term matrix
is block-diagonal over categories (c_i == c_j mask), so sharding BY CATEGORY
makes every core independent.  16 categories over 8 cores = 2/core, paired
large-with-small (sorted counts, pair (k, 15-k)) so the padded per-slot sizes
(P0, P1) and total rows R = P0 + P1 are minimal.

Per core (layouts are [feature, row] so matmuls contract along partitions):
  h   = W1^T x          (fp16, 2 k-chunks, PSUM f32)
  ht  = relu(h + b1)    (fp16, relu split across Scalar/Vector engines)
  fz  = Wz^T z + bz     (fp16)
  u   = W2c[g]^T ht + b2c[g]   per category, W2c = W2 @ w_s[g] host-fused
  pm  = u_chunk^T fz    per (category, 128-row chunk)  [128, P_g] PSUM
  pos = diag(pm)        via identity-mask tensor_tensor_reduce (Vector)
  nacc= sum_j relu(pm)  row-wise, alternating Vector / Scalar(ACT accum)
  out = log(softplus(pos)+eps) - log(nacc/cnt + eps)

Numerics: softplus ~= relu in the negative sum (per-row |v| std >= 10, bias
<= ~1e-3 on the output); the positive term uses an exact piecewise log form;
padded rows get z := z0 with Wz^T z0 + bz = 0 (host-solved against fp16 Wz)
so they contribute ~nothing to the relu-sum; counts use the true 1/cnt.

Perf notes vs v1: inputs ship in 3 packed dram blobs (6 DMA issues instead of
~25, so the DMA streams start at the preamble end instead of being serialized
behind warmup/WAR hazards); the PE warmup buffer lives in the persistent pool
so input DMAs are not blocked; all ACT functions (Relu/Identity/Abs/Exp/Ln)
live in one table set (natural_log_exp_and_others) so there is no mid-kernel
1.3us table reload; one rotating PSUM tag (2 bufs x 3 banks); the fp32r
positive-term matmuls + f32 prod pass of v1 are replaced by diag extraction.
"""

import math
from contextlib import ExitStack

import numpy as np

import concourse.bass as bass
import concourse.mybir as mybir
import concourse.tile as tile
from concourse import bacc
from concourse import bass_utils

F32 = mybir.dt.float32
BF16 = mybir.dt.bfloat16
FP16 = mybir.dt.float16
AF = mybir.ActivationFunctionType
ALU = mybir.AluOpType

N, D_IN, HID, Z, C = 8192, 256, 512, 128, 16
N_CORES = 8
CATS_PER_CORE = C // N_CORES
EPS32 = float(np.float32(1e-16))
LNEPS = float(np.log(np.float64(np.float32(1e-16))))  # -36.8413614...
POS_THRESH = -9.0
N_WARMUP_MM = 8


def _col_tiles(total, step=512):
    tiles = []
    s = 0
    while s < total:
        nt = min(step, total - s)
        tiles.append((s, nt))
        s += nt
    return tiles


def build_program(P0, P1):
    NCH = [P0 // 128, P1 // 128]
    R = P0 + P1
    F = NCH[0] + NCH[1]
    GOFF = [0, P0]  # row offset of each slot
    PS = [P0, P1]
    COLB = [0, NCH[0]]  # chunk-major column base per slot
    RTIL = _col_tiles(R)

    # wblob fp16 column offsets
    W1_OFF = 0            # [2, 512]  cols f*512 + hcol
    W2C_OFF = 1024        # [2, 4, 128] cols g*512 + q*128 + j
    WZ_OFF = 2048         # [128]
    WCOLS = 2176
    # cblob f32 column offsets
    B1_OFF = 0            # 4
    B2C_OFF = 4           # 2
    BZ_OFF = 6            # 1
    INV_OFF = 7           # F
    EPS_OFF = 7 + F       # 1
    ID_OFF = 8 + F        # [128] identity f32
    CCOLS = 8 + F + 128

    nc = bacc.Bacc(
        "TRN2",
        target_bir_lowering=False,
        debug=False,
        enable_asserts=False,
        num_devices=N_CORES,
    )

    xz = nc.dram_tensor("xz", [128, 3 * R], FP16, kind="ExternalInput")
    wb = nc.dram_tensor("wb", [128, WCOLS], FP16, kind="ExternalInput")
    cb = nc.dram_tensor("cb", [128, CCOLS], F32, kind="ExternalInput")
    outd = nc.dram_tensor("out", [128, F], F32, kind="ExternalOutput")

    with tile.TileContext(nc) as tc, ExitStack() as ctx:
        perm = ctx.enter_context(tc.tile_pool(name="perm", bufs=1))
        ps = ctx.enter_context(tc.tile_pool(name="ps", bufs=1, space="PSUM"))

        # ---- persistent SBUF ----
        sbW = perm.tile([128, WCOLS], FP16)
        sbC = perm.tile([128, CCOLS], F32)
        sbXZ = perm.tile([128, 3 * R], FP16)
        ht = perm.tile([128, 4, R], FP16)
        fz16 = perm.tile([128, R], FP16)
        u16 = perm.tile([128, R], FP16)
        nacc = perm.tile([128, F], F32)
        posT = perm.tile([128, F], F32)
        junkV = perm.tile([128, P0], FP16)
        junkS = perm.tile([128, P0], FP16)
        junkD = perm.tile([128, 128], F32)
        wmov = perm.tile([128, 256], BF16)

        def wcol(off, n):
            return sbW[:, off : off + n]

        def ccol(off, n=1):
            return sbC[:, off : off + n]

        # ---- DMA issues: scalar carries weights/consts, sync carries x/z.
        # W1 first (first MLP matmuls need only W1 + x f=0).
        nc.scalar.dma_start(sbW[:, 0:1024], wb[:, 0:1024])
        nc.scalar.dma_start(sbW[:, 1024:WCOLS], wb[:, 1024:WCOLS])
        nc.scalar.dma_start(sbC[:], cb[:])
        nc.sync.dma_start(sbXZ[:, 0:R], xz[:, 0:R])
        nc.sync.dma_start(sbXZ[:, R : 2 * R], xz[:, R : 2 * R])
        nc.sync.dma_start(sbXZ[:, 2 * R : 3 * R], xz[:, 2 * R : 3 * R])

        # ---- PE warmup: engage the HAM clock boost while DMAs land.
        # Buffers live in the persistent pool, so nothing downstream aliases
        # them and the input DMAs are never blocked behind the warmup reads.
        nc.gpsimd.memset(wmov[:], 0.5)
        pwarm = ps.tile([16, 256], F32, tag="warm")
        for _ in range(N_WARMUP_MM):
            nc.tensor.matmul(
                pwarm[:], wmov[:, 0:16], wmov[:], start=True, stop=True
            )

        xv = [sbXZ[:, 0:R], sbXZ[:, R : 2 * R]]
        zv = sbXZ[:, 2 * R : 3 * R]

        # ======== MLP layer 1: h-chunk major, f-outer (8 LDWEIGHTS) ========
        relu_eng = 0
        for h in range(4):
            ph = ps.tile([128, R], F32, tag="big", name=f"ph{h}", bufs=2)
            for (ts, nt) in RTIL:
                for f in range(2):
                    w1 = wcol(W1_OFF + f * 512 + h * 128, 128)
                    nc.tensor.matmul(
                        ph[:, ts : ts + nt],
                        w1,
                        xv[f][:, ts : ts + nt],
                        start=(f == 0),
                        stop=(f == 1),
                    )
            b1h = ccol(B1_OFF + h)
            if relu_eng == 0:
                nc.scalar.activation(ht[:, h, :], ph[:], AF.Relu, bias=b1h)
            else:
                nc.vector.tensor_scalar(
                    ht[:, h, :], ph[:], b1h, 0.0, op0=ALU.add, op1=ALU.max
                )
            relu_eng ^= 1

        # ======== f_z = Wz^T z + bz ========
        pfz = ps.tile([128, R], F32, tag="big", name="pfz", bufs=2)
        wz = wcol(WZ_OFF, 128)
        for (ts, nt) in RTIL:
            nc.tensor.matmul(
                pfz[:, ts : ts + nt], wz, zv[:, ts : ts + nt], start=True, stop=True
            )
        nc.scalar.activation(fz16[:], pfz[:], AF.Identity, bias=ccol(BZ_OFF))

        # ======== u = W2c[g]^T ht + b2c[g] per slot ========
        for g in range(2):
            pu = ps.tile([128, R], F32, tag="big", name=f"pu{g}", bufs=2)
            for (ts, nt) in _col_tiles(PS[g]):
                for q in range(4):
                    w2 = wcol(W2C_OFF + g * 512 + q * 128, 128)
                    nc.tensor.matmul(
                        pu[:, ts : ts + nt],
                        w2,
                        ht[:, q, GOFF[g] + ts : GOFF[g] + ts + nt],
                        start=(q == 0),
                        stop=(q == 3),
                    )
            nc.vector.tensor_scalar_add(
                u16[:, GOFF[g] : GOFF[g] + PS[g]],
                pu[:, 0 : PS[g]],
                ccol(B2C_OFF + g),
            )

        # ======== stage C: pm = u_chunk^T fz; pos = diag; nacc = relu-sum ====
        ident = ccol(ID_OFF, 128)
        k = 0
        for g in range(2):
            fzg = fz16[:, GOFF[g] : GOFF[g] + PS[g]]
            for ic in range(NCH[g]):
                c0 = GOFF[g] + ic * 128
                pm = ps.tile([128, R], F32, tag="big", name=f"pm{g}_{ic}", bufs=2)
                for (ts, nt) in _col_tiles(PS[g]):
                    nc.tensor.matmul(
                        pm[:, ts : ts + nt],
                        u16[:, c0 : c0 + 128],
                        fzg[:, ts : ts + nt],
                        start=True,
                        stop=True,
                    )
                col = COLB[g] + ic
                # pos[:, col] = diag of pm's own-chunk block
                nc.vector.scalar_tensor_tensor(
                    junkD[:],
                    pm[:, ic * 128 : ic * 128 + 128],
                    0.0,
                    ident,
                    op0=ALU.add,
                    op1=ALU.mult,
                    accum_out=posT[:, col : col + 1],
                )
                if k % 2 == 0:
                    nc.scalar.activation(
                        junkS[:, 0 : PS[g]],
                        pm[:, 0 : PS[g]],
                        AF.Relu,
                        accum_out=nacc[:, col : col + 1],
                    )
                else:
                    nc.vector.tensor_scalar(
                        junkV[:, 0 : PS[g]],
                        pm[:, 0 : PS[g]],
                        0.0,
                        0.0,
                        op0=ALU.max,
                        op1=ALU.add,
                        accum_out=nacc[:, col : col + 1],
                    )
                k += 1

        # ======== positive-term piecewise log(softplus(pos)+eps) ========
        # All ACT funcs here (Abs/Exp/Ln) share one table set with Relu.
        vec = perm
        t_ax = vec.tile([128, F], F32)
        nc.scalar.activation(t_ax[:], posT[:], AF.Abs)
        t_y = vec.tile([128, F], F32)
        nc.vector.tensor_scalar_add(t_y[:], posT[:], -LNEPS)
        t_ay = vec.tile([128, F], F32)
        nc.scalar.activation(t_ay[:], t_y[:], AF.Abs)
        t_e2 = vec.tile([128, F], F32)
        nc.scalar.activation(t_e2[:], t_ax[:], AF.Exp, scale=-1.0)
        t_e1 = vec.tile([128, F], F32)
        nc.scalar.activation(t_e1[:], t_ay[:], AF.Exp, scale=-1.0)
        t_r2 = vec.tile([128, F], F32)
        nc.vector.tensor_scalar_max(t_r2[:], posT[:], 0.0)
        t_r1 = vec.tile([128, F], F32)
        nc.vector.tensor_scalar_max(t_r1[:], t_y[:], 0.0)
        t_l2 = vec.tile([128, F], F32)
        nc.scalar.activation(t_l2[:], t_e2[:], AF.Ln, bias=1.0)
        t_l1 = vec.tile([128, F], F32)
        nc.scalar.activation(t_l1[:], t_e1[:], AF.Ln, bias=1.0)
        t_sp = vec.tile([128, F], F32)
        nc.vector.tensor_add(t_sp[:], t_r2[:], t_l2[:])
        t_p2 = vec.tile([128, F], F32)
        nc.scalar.activation(t_p2[:], t_sp[:], AF.Ln, bias=ccol(EPS_OFF))
        t_p1 = vec.tile([128, F], F32)
        nc.vector.scalar_tensor_tensor(
            t_p1[:], t_r1[:], LNEPS, t_l1[:], op0=ALU.add, op1=ALU.add
        )
        t_m = vec.tile([128, F], mybir.dt.int32)
        nc.vector.tensor_scalar(t_m[:], posT[:], POS_THRESH, None, op0=ALU.is_lt)
        t_posln = vec.tile([128, F], F32)
        nc.vector.select(t_posln[:], t_m[:], t_p1[:], t_p2[:])

        # ======== final combination ========
        t_negT = vec.tile([128, F], F32)
        nc.vector.tensor_mul(t_negT[:], nacc[:], ccol(INV_OFF, F))
        t_lnneg = vec.tile([128, F], F32)
        nc.scalar.activation(t_lnneg[:], t_negT[:], AF.Ln, bias=ccol(EPS_OFF))
        t_out = vec.tile([128, F], F32)
        nc.vector.tensor_sub(t_out[:], t_posln[:], t_lnneg[:])
        nc.sync.dma_start(outd[:], t_out[:])

    nc.compile()
    return nc


def prepare(x, c, z, W1, b1, W2, b2, Wz, bz, w_s):
    """Host-side sharding: returns (P0, P1, in_maps, slots, idx)."""
    x = np.ascontiguousarray(np.asarray(x, dtype=np.float32))
    z = np.ascontiguousarray(np.asarray(z, dtype=np.float32))
    W1 = np.asarray(W1, dtype=np.float32)
    b1 = np.asarray(b1, dtype=np.float32)
    W2 = np.asarray(W2, dtype=np.float32)
    b2 = np.asarray(b2, dtype=np.float32)
    Wz = np.asarray(Wz, dtype=np.float32)
    bz = np.asarray(bz, dtype=np.float32)
    w_s = np.asarray(w_s, dtype=np.float32)
    ci = np.asarray(c).astype(np.int64)

    idx = [np.nonzero(ci == g)[0] for g in range(C)]
    cnt = np.array([len(i) for i in idx])
    order = np.argsort(-cnt)  # descending
    # core k gets (order[k], order[15-k]): biggest with smallest
    pairs = [(int(order[k]), int(order[C - 1 - k])) for k in range(N_CORES)]
    P0 = 128 * max(1, math.ceil(max(cnt[p[0]] for p in pairs) / 128))
    P1 = 128 * max(1, math.ceil(max(cnt[p[1]] for p in pairs) / 128))
    PS = [P0, P1]
    NCH = [P0 // 128, P1 // 128]
    R = P0 + P1
    F = NCH[0] + NCH[1]

    # padded rows get z0 with Wz^T z0 + bz = 0 (device uses fp16 Wz)
    z0 = -np.linalg.solve(
        Wz.astype(np.float16).astype(np.float64).T, bz.astype(np.float64)
    ).astype(np.float32)

    # W2c[g] = W2 @ w_s[g], b2c[g] = b2 @ w_s[g]
    W2c_all = np.einsum(
        "hd,cde->che", W2.astype(np.float64), w_s.astype(np.float64)
    )  # [C, HID, Z]
    b2c_all = np.einsum(
        "d,cde->ce", b2.astype(np.float64), w_s.astype(np.float64)
    )  # [C, Z]

    # weight blob (identical on all cores except W2c slots)
    wb_base = np.zeros((128, 2176), dtype=np.float16)
    wb_base[:, 0:512] = W1[0:128, :].astype(np.float16)
    wb_base[:, 512:1024] = W1[128:256, :].astype(np.float16)
    wb_base[:, 2048:2176] = Wz.astype(np.float16)

    cb_base = np.zeros((128, 8 + F + 128), dtype=np.float32)
    cb_base[:, 0:4] = b1.reshape(4, 128).T
    cb_base[:, 6] = bz
    cb_base[:, 7 + F] = EPS32
    cb_base[:, 8 + F : 8 + F + 128] = np.eye(128, dtype=np.float32)

    in_maps = []
    slots = []
    for k in range(N_CORES):
        cats = pairs[k]
        rows = []
        pad_flags = np.zeros(R, dtype=bool)
        wbk = wb_base.copy()
        cbk = cb_base.copy()
        off = 0
        for j, g in enumerate(cats):
            n_real = cnt[g]
            pad_to = PS[j] - n_real
            fill = idx[g][0] if n_real > 0 else 0
            rows.append(
                np.concatenate([idx[g], np.full(pad_to, fill, dtype=np.int64)])
            )
            pad_flags[off + n_real : off + PS[j]] = True
            colb = 0 if j == 0 else NCH[0]
            cbk[:, 7 + colb : 7 + colb + NCH[j]] = 1.0 / max(n_real, 1)
            wbk[:, 1024 + j * 512 : 1024 + (j + 1) * 512] = (
                W2c_all[g].reshape(4, 128, 128).transpose(1, 0, 2).reshape(128, 512)
            ).astype(np.float16)
            cbk[:, 4 + j] = b2c_all[g].astype(np.float32)
            off += PS[j]
        rows = np.concatenate(rows)  # [R]
        xk = x[rows]  # [R, 256]
        zk = z[rows].copy()
        zk[pad_flags] = z0.reshape(-1)
        xzk = np.zeros((128, 3 * R), dtype=np.float16)
        xT = xk.T.astype(np.float16)  # [256, R]
        xzk[:, 0:R] = xT[0:128]
        xzk[:, R : 2 * R] = xT[128:256]
        xzk[:, 2 * R : 3 * R] = zk.T.astype(np.float16)
        in_maps.append({"xz": xzk, "wb": wbk, "cb": cbk})
        slots.append((cats, [int(cnt[g]) for g in cats]))
    return P0, P1, in_maps, slots, idx


def gather_output(P0, P1, slots, idx, core_outs):
    NCH = [P0 // 128, P1 // 128]
    out_full = np.zeros(N, dtype=np.float32)
    for k in range(N_CORES):
        om = core_outs[k]  # [128, F]; out[p, colb+ic] = row off + ic*128 + p
        cats, counts = slots[k]
        colb = 0
        for j, g in enumerate(cats):
            nch = NCH[j]
            rows_cat = om[:, colb : colb + nch].T.reshape(128 * nch)
            if counts[j]:
                out_full[idx[g]] = rows_cat[: counts[j]]
            colb += nch
    return out_full


def kernel(x, c, z, W1, b1, W2, b2, Wz, bz, w_s):
    P0, P1, in_maps, slots, idx = prepare(x, c, z, W1, b1, W2, b2, Wz, bz, w_s)
    nc = build_program(P0, P1)
    res = bass_utils.run_bass_kernel_spmd(nc, in_maps, core_ids=list(range(N_CORES)))
    return gather_output(P0, P1, slots, idx, [r["out"] for r in res.results])
